# revision 2
# baseline (speedup 1.0000x reference)
"""8-device ConvLSTM: B(4) x H-halves(2), communication-free via
redundant-overlap decay in 4 chunks of 16 steps.

Device d = (b, s): batch b = d//2, half s = d%2. Odd halves are fed
vertically flipped input + flipped kernels so all devices run the same
program (top half semantics). Each chunk k (steps 16k..16k+15) computes a
static row extent R_k = 128 + min(126, 2*(64 - 16k)) (+2 conv halo rows
consumed per step internally).

Correctness: at entry to chunk k the carry (h, c) is valid on rows
[0, R_k_prev_exit) where validity shrinks by 2 rows/step inside a chunk;
chunk k computes on rows [0, R_k) and exits with validity >= R_{k+1}+32
... >= what chunk k+1 needs. Final step validity covers rows [0,128).
"""

import numpy as np
import jax
import jax.numpy as jnp
from functools import partial

D = 64
H = 256
W = 256
HALF = 128
NCHUNK = 4
CSTEPS = D // NCHUNK  # 16


def _conv2d(x, w):
    return jax.lax.conv_general_dilated(
        x, w, window_strides=(1, 1), padding=((2, 2), (2, 2)),
        dimension_numbers=("NCHW", "OIHW", "NCHW"))


def _rows_for_chunk(k):
    # rows needed at the START of chunk k so that after 16 steps of
    # 2-row/step decay, rows [0, rows_for_chunk(k+1)) are still valid.
    r = HALF + 2 * (D - CSTEPS * k)
    return min(H, r)


@partial(jax.pmap, axis_name="d")
def _scan_half(x_d, Wx, Wh, b):
    # x_d: [1, 1, D, H, W] (already flipped for odd halves); weights per-device.
    bias = b.reshape(1, 4, 1, 1)

    def step(carry, x_t):
        h_prev, c_prev = carry
        z = jax.nn.relu(_conv2d(x_t, Wx)) + jax.nn.relu(_conv2d(h_prev, Wh)) + bias
        i_g = jax.nn.sigmoid(z[:, 0:1])
        c_g = jnp.tanh(z[:, 2:3])
        o_g = jax.nn.sigmoid(z[:, 3:4])
        c_now = (c_g + c_prev) * i_g
        h_now = o_g * jnp.tanh(c_now)
        return (h_now, c_now), h_now

    R0 = _rows_for_chunk(0)
    h = jnp.zeros((1, 1, R0, W), dtype=x_d.dtype)
    c = jnp.zeros((1, 1, R0, W), dtype=x_d.dtype)
    outs = []
    for k in range(NCHUNK):
        Rk = _rows_for_chunk(k)
        h = h[:, :, :Rk]
        c = c[:, :, :Rk]
        x_seq = jnp.moveaxis(x_d[:, :, CSTEPS * k : CSTEPS * (k + 1), :Rk], 2, 0)
        (h, c), hs = jax.lax.scan(step, (h, c), x_seq)
        outs.append(hs[:, :, :, :HALF])  # [16, 1, 1, 128, W]
    return jnp.concatenate(outs, axis=0)  # [D, 1, 1, 128, W]


def kernel(x, Wx, Wh, b, direction):
    x = np.asarray(x, dtype=np.float32)
    B = x.shape[0]
    Wx = np.asarray(Wx, np.float32)
    Wh = np.asarray(Wh, np.float32)
    b = np.asarray(b, np.float32)

    # build per-device inputs: 8 devices = (b, s)
    xs = np.empty((2 * B, 1, 1, D, H, W), dtype=np.float32)
    Wxs = np.empty((2 * B, 4, 1, 5, 5), dtype=np.float32)
    Whs = np.empty_like(Wxs)
    for bb in range(B):
        xs[2 * bb, 0] = x[bb]  # top half device sees rows as-is
        xs[2 * bb + 1, 0] = x[bb, :, :, ::-1, :]  # bottom half flipped
        Wxs[2 * bb] = Wx
        Whs[2 * bb] = Wh
        Wxs[2 * bb + 1] = Wx[:, :, ::-1, :]
        Whs[2 * bb + 1] = Wh[:, :, ::-1, :]
    bs = np.broadcast_to(b, (2 * B, 4)).copy()

    out = np.asarray(_scan_half(xs, Wxs, Whs, bs))  # [8, D, 1, 1, 128, W]
    res = np.empty((B, 1, D, H, W), dtype=np.float32)
    for bb in range(B):
        res[bb, 0, :, :HALF] = out[2 * bb, :, 0, 0]
        res[bb, 0, :, HALF:] = out[2 * bb + 1, :, 0, 0][:, ::-1, :]
    return res


def _np_conv2d(img, w4):
    # img [H, W]; w4 [4, 5, 5] -> [4, H, W], SAME zero pad
    Hh, Ww = img.shape
    pad = np.zeros((Hh + 4, Ww + 4), dtype=np.float32)
    pad[2:-2, 2:-2] = img
    out = np.zeros((4, Hh, Ww), dtype=np.float32)
    for c in range(4):
        for dy in range(5):
            for dx in range(5):
                out[c] += w4[c, dy, dx] * pad[dy : dy + Hh, dx : dx + Ww]
    return out


def _np_ref_steps(x_b, Wx, Wh, b, nsteps):
    # x_b [D, H, W] single batch; returns h for steps 0..nsteps-1 [nsteps, H, W]
    Hh, Ww = x_b.shape[1:]
    h = np.zeros((Hh, Ww), np.float32)
    c = np.zeros((Hh, Ww), np.float32)
    bias = b.reshape(4, 1, 1)
    outs = []
    for t in range(nsteps):
        z = (np.maximum(_np_conv2d(x_b[t], Wx[:, 0]), 0)
             + np.maximum(_np_conv2d(h, Wh[:, 0]), 0) + bias)
        i_g = 1 / (1 + np.exp(-z[0]))
        c_g = np.tanh(z[2])
        o_g = 1 / (1 + np.exp(-z[3]))
        c = (c_g + c) * i_g
        h = o_g * np.tanh(c)
        outs.append(h.copy())
    return np.stack(outs)


if __name__ == "__main__":
    rng = np.random.default_rng(0)
    x = rng.standard_normal((4, 1, 64, 256, 256), dtype=np.float32)
    Wx = (rng.standard_normal((4, 1, 5, 5)) * 0.1).astype(np.float32)
    Wh = (rng.standard_normal((4, 1, 5, 5)) * 0.1).astype(np.float32)
    b = (rng.standard_normal(4) * 0.1).astype(np.float32)
    out = kernel(x=x, Wx=Wx, Wh=Wh, b=b, direction=0)
    print(out.shape, out.dtype, float(np.abs(out).mean()))
    import time
    t0 = time.perf_counter()
    kernel(x=x, Wx=Wx, Wh=Wh, b=b, direction=0)
    print(f"second call: {time.perf_counter() - t0:.3f}s")
    # numpy check: batch 0, first 20 steps (covers chunk boundary at 16)
    NS = 20
    ref = _np_ref_steps(x[0, 0], Wx, Wh, b, NS)
    got = out[0, 0, :NS]
    err = np.linalg.norm(got - ref) / np.linalg.norm(ref)
    print(f"numpy-check rel err (20 steps, b0): {err:.3e}")
